# revision 16
# baseline (speedup 1.0000x reference)
"""Trainium2 Bass kernel for nn_NetBurgers1D_InN_legendre.

Strategy
--------
Data-parallel over batch: 8 cores x 4 samples each. All on-chip compute uses a
"phase-major" layout: SBUF row = phase*20 + channel, where phase = t mod 6 and
the free dim indexes 6-sample blocks u (t = 6u + phase). In this layout every
op of the network becomes a dense matmul on the 120-partition contraction dim:

 - first conv (10ch k=3)            : 1 matmul, lhsT (80, 120), rhs = 8-phase
                                      de-interleaved input (built on host)
 - residual conv3 -> gelu -> conv3  : 2 + 2 matmuls (block-shift taps), built
                                      with output shifts so the final add is a
                                      pure PSUM accumulation
 - Legendre decompose/mix/reconstruct: folded on host into three 120x120
                                      block-diagonal matrices (3 matmuls)
 - w11 (20->128, per phase)         : 6 zero-masked (120,128) matmuls
 - w_out (128->1, per phase)        : 6 accumulating (128,6) matmuls writing
                                      each phase to its own output partition
Matmuls run as float32r (full-rate PE). Gelu on ScalarE reading PSUM directly;
the rec + linear_x add happens inside PSUM accumulation for free.

Host pre/post: wrap-pad + 8-phase de-interleave of the input, folding of all
weights into lhsT matrices, and re-interleave of the (6, nblk) phase-major
output back to (1, 16384).
"""

import sys

sys.path.insert(0, "/opt/trn_rl_repo")

import numpy as np

import concourse.bacc as bacc
import concourse.mybir as mybir
import concourse.tile as tile
from concourse.bass_utils import run_bass_kernel_spmd

# ---------------------------------------------------------------- constants
B, IN_LEN, L = 32, 10, 16384
N_POLY, M_MODES, K_STR, CH, NB = 12, 6, 2, 20, 4
S = N_POLY // K_STR        # 6: block size (= conv stride)
RECEPT = 25
R_PAD = 2
LPAD = L + 2 * RECEPT + R_PAD   # 16436
NBLK0 = (LPAD - 2) // S         # 2739 blocks after the first conv
NBLK4 = NBLK0 - 2 * NB          # 2731 blocks after the 4 residual blocks
# fp32r matmuls need even N, and shifted taps read past the computed range,
# so every stage computes a few junk-but-initialized spare columns. Widths
# cascade backward from the output: final reads 2732, block i consumes 2 more
# per tap level. The input is zero-padded to NBLKW on the host.
NBLKW = NBLK0 + 9               # 2748: tile width / input width
ATOT = [2746 - 4 * i for i in range(NB)]   # a-stage computed cols, block i
OTOT = [2744 - 4 * i for i in range(NB)]   # o-stage computed cols, block i
FTOT = 2732                                # final-stage computed cols
N_CORES = 8
BPC = B // N_CORES              # 4 samples per core
P = S * CH                      # 120 partitions in phase-major layout
HID = 128                       # w11 hidden width
CHUNK = 512                     # PSUM bank = 512 fp32

F32 = mybir.dt.float32
F32R = mybir.dt.float32r
F16 = mybir.dt.float16
GELU = mybir.ActivationFunctionType.Gelu

# weight wall slots: 1 first conv + 7 per block * 4 + 6 w11 + 6 w_out
N_WSLOT = 1 + 7 * NB + 2 * S
WCOL = 128


# ------------------------------------------------------------- host folding
def _conv_pair(w, shift):
    """(T0, T1) lhsT for conv3 with output shift; row jj*20+ci, col j*20+co."""
    cout, cin, kw = w.shape
    T0 = np.zeros((S * cin, S * cout), np.float32)
    T1 = np.zeros((S * cin, S * cout), np.float32)
    for j in range(S):
        for k in range(kw):
            jj = j + shift + k
            T = T0 if jj < S else T1
            T[(jj % S) * cin:(jj % S) * cin + cin,
              j * cout:j * cout + cout] += w[:, :, k].T
    return T0, T1


def _first_conv_mat(w_first):
    W1 = np.zeros((8 * IN_LEN, P), np.float32)
    for j in range(S):
        for k in range(3):
            jj = j + k
            W1[jj * IN_LEN:(jj + 1) * IN_LEN, j * CH:(j + 1) * CH] += \
                w_first[:, :, k].T
    return W1


def _legendre_mats(lin_m_i, fd, fr):
    """Fold decompose -> (mode-major grouped) mix -> reconstruct into three
    120x120 lhsT tap matrices.

    The reference flattens (mode, channel) mode-major and applies the grouped
    1x1 conv over *that* axis in chunks of 6 -- the groups straddle channels.
    So: D maps phase-major input blocks to the mode-major 120-vector, M is
    block-diagonal in mode-major flat index, R maps back per channel.
    """
    Fd0 = fd[:, 0, :S] / K_STR   # (mode, in-phase)
    Fd1 = fd[:, 0, S:] / K_STR
    Fr0 = fr[:, 0, :S] / K_STR   # (mode, out-phase)
    Fr1 = fr[:, 0, S:] / K_STR
    D0 = np.zeros((P, P), np.float32)  # [mode*20+ch, jj*20+ci]
    D1 = np.zeros_like(D0)
    R0 = np.zeros((P, P), np.float32)  # [j*20+co, mode*20+ch]
    R1 = np.zeros_like(R0)
    for ch in range(CH):
        for mode in range(M_MODES):
            f = mode * CH + ch
            for j in range(S):
                D0[f, j * CH + ch] = Fd0[mode, j]
                D1[f, j * CH + ch] = Fd1[mode, j]
                R0[j * CH + ch, f] = Fr0[mode, j]
                R1[j * CH + ch, f] = Fr1[mode, j]
    M = np.zeros((P, P), np.float32)   # mode-major block-diag groups of 6
    for g in range(CH):
        M[S * g:S * g + S, S * g:S * g + S] = lin_m_i[g]   # [o, i]
    T0 = 2.0 * (R1 @ M @ D0)           # [out, in] for tap u'
    T1 = 2.0 * (R1 @ M @ D1 + R0 @ M @ D0)
    T2 = 2.0 * (R0 @ M @ D1)
    return T0.T, T1.T, T2.T            # lhsT convention [in, out]


def _build_wall(w_first, conv_a, conv_b, lin_m, w11, w_out, filt_d, filt_r):
    """Pack every lhsT into one (128, N_WSLOT*128) array, slot i at col i*128."""
    wall = np.zeros((HID, N_WSLOT * WCOL), np.float32)
    W1 = _first_conv_mat(w_first)
    mats = [np.zeros((1, 1), np.float32)]   # slot 0 unused (conv folded away)
    for i in range(NB):
        A0, A1 = _conv_pair(conv_a[i], 3)
        B0, B1 = _conv_pair(conv_b[i], 1)
        C0, C1, C2 = _legendre_mats(lin_m[i], filt_d, filt_r)
        if i == 0:
            # block 0 reads the 8-phase input directly: x0 = W1^T in80, and
            # every block-0 lhsT T becomes W1 @ T (conv_b reads `a`, unchanged)
            A0, A1, C0, C1, C2 = (W1 @ A0, W1 @ A1, W1 @ C0, W1 @ C1, W1 @ C2)
        mats += [A0, A1, B0, B1, C0, C1, C2]
    for j in range(S):  # w11, zero-masked to phase j: (120, 128)
        Wz = np.zeros((P, HID), np.float32)
        Wz[j * CH:(j + 1) * CH, :] = w11[:, :, 0].T
        mats.append(Wz)
    for j in range(S):  # w_out into output partition j: (128, 6)
        Wz = np.zeros((HID, S), np.float32)
        Wz[:, j] = w_out[0, :, 0]
        mats.append(Wz)
    for i, m in enumerate(mats):
        wall[:m.shape[0], i * WCOL:i * WCOL + m.shape[1]] = m
    return wall


def _make_in80(xpad):
    """(b, 10, 16436) -> (b, 80, 2748): row jj*10+ci holds xpad[ci, 6u+jj].

    Columns >= NBLK0 (the spare tail) are zero."""
    b = xpad.shape[0]
    out = np.zeros((b, 8 * IN_LEN, NBLKW), np.float32)
    for jj in range(8):
        out[:, jj * IN_LEN:(jj + 1) * IN_LEN, :NBLK0] = \
            xpad[:, :, jj:jj + S * NBLK0:S][:, :, :NBLK0]
    return out


def _even(n):
    return n + (n & 1)


def _chunks(n):
    """Balanced even-size chunks of n (n even) columns, each <=CHUNK, >=256."""
    assert n % 2 == 0
    k = (n + CHUNK - 1) // CHUNK
    base = (n // k) & ~1
    rem = (n - base * k) // 2
    out = []
    c0 = 0
    for i in range(k):
        cs = base + (2 if i < rem else 0)
        out.append((c0, cs))
        c0 += cs
    assert c0 == n
    return out


# ------------------------------------------------------------- bass kernel
def _emit(nc):
    in_d = nc.dram_tensor("in80", [BPC, 8 * IN_LEN, NBLKW], F16,
                          kind="ExternalInput")
    wall_d = nc.dram_tensor("wall", [HID, N_WSLOT * WCOL], F16,
                            kind="ExternalInput")
    out_d = nc.dram_tensor("yout", [BPC, S, NBLK4], F32, kind="ExternalOutput")

    def wslot(i, k, m):
        return wall_sb[:k, i * WCOL:i * WCOL + m]

    with tile.TileContext(nc) as tc:
        with (
            tc.tile_pool(name="wsb", bufs=1) as wsb,
            tc.tile_pool(name="xin", bufs=2) as xin,
            tc.tile_pool(name="xbuf", bufs=6) as xbuf,
            tc.tile_pool(name="abuf", bufs=2) as abuf,
            tc.tile_pool(name="gbuf", bufs=8) as gbuf,
            tc.tile_pool(name="obuf", bufs=2) as obuf,
            tc.tile_pool(name="psA", bufs=2, space="PSUM") as psA,
            tc.tile_pool(name="psO", bufs=2, space="PSUM") as psO,
            tc.tile_pool(name="psH", bufs=2, space="PSUM") as psH,
            tc.tile_pool(name="psY", bufs=2, space="PSUM") as psY,
        ):
            wall_sb = wsb.tile([HID, N_WSLOT * WCOL], F16)
            nc.sync.dma_start(out=wall_sb[:, :8 * WCOL],
                              in_=wall_d.ap()[:, :8 * WCOL])
            nc.sync.dma_start(out=wall_sb[:, 8 * WCOL:],
                              in_=wall_d.ap()[:, 8 * WCOL:])

            def emit_a_pair(W, kk, xt, a, pair):
                """Chunk-pair a-stage, weight-outer: each lhsT is loaded once
                and streamed into both chunks' psum banks before swapping."""
                A0, A1 = W
                pas = [psA.tile([HID, CHUNK], F32, tag="psA", name="pa")
                       for _ in pair]
                for w, sh, first, last in ((A0, 0, True, False),
                                           (A1, 1, False, True)):
                    for pa, (c0, cs) in zip(pas, pair):
                        nc.tensor.matmul(pa[:, :cs], w[:kk],
                                         xt[:, c0 + sh:c0 + sh + cs],
                                         start=first, stop=last)
                for pa, (c0, cs) in zip(pas, pair):
                    nc.scalar.activation(out=a[:, c0:c0 + cs], in_=pa[:P, :cs],
                                         func=GELU)

            def emit_o_pair(W, kk, xt, a, xn, pair):
                pos = [psO.tile([HID, CHUNK], F32, tag="psO", name="po")
                       for _ in pair]
                taps = ((W[0], a, 0, True, False), (W[1], a, 1, False, False),
                        (W[2], xt, 0, False, False), (W[3], xt, 1, False, False),
                        (W[4], xt, 2, False, True))
                for w, src, sh, first, last in taps:
                    ww = w[:kk] if src is xt else w
                    for po, (c0, cs) in zip(pos, pair):
                        nc.tensor.matmul(po[:, :cs], ww,
                                         src[:, c0 + sh:c0 + sh + cs],
                                         start=first, stop=last)
                for po, (c0, cs) in zip(pos, pair):
                    nc.scalar.activation(out=xn[:, c0:c0 + cs], in_=po[:P, :cs],
                                         func=GELU)

            def block_emissions(s, i, xt):
                """Generator of chunk-pair emission thunks for residual block i,
                interleaving a/o pairs so PE-heavy o work hides a-gelu latency."""
                kk = 8 * IN_LEN if i == 0 else P
                A = (wslot(1 + 7 * i + 0, P, HID),
                     wslot(1 + 7 * i + 1, P, HID))
                O = tuple(wslot(1 + 7 * i + t, P, HID) for t in range(2, 7))
                a = abuf.tile([P, NBLKW], F16, tag="a", name="a")
                xn = xbuf.tile([P, NBLKW], F16, tag="x", name="xn")
                a_ch = _chunks(ATOT[i])
                o_ch = _chunks(OTOT[i])
                a_pairs = [a_ch[k:k + 2] for k in range(0, len(a_ch), 2)]
                o_pairs = [o_ch[k:k + 2] for k in range(0, len(o_ch), 2)]
                yield (lambda p=a_pairs[0]: emit_a_pair(A, kk, xt, a, p)), xn
                for k, op in enumerate(o_pairs):
                    if k + 1 < len(a_pairs):
                        yield (lambda p=a_pairs[k + 1]:
                               emit_a_pair(A, kk, xt, a, p)), xn
                    yield (lambda p=op: emit_o_pair(O, kk, xt, a, xn, p)), xn

            def final_emissions(s, x4):
                out_t = obuf.tile([S, FTOT], F32, tag="out", name="out_t")

                def emit_final_pair(pair):
                    pys = [psY.tile([S, CHUNK], F32, tag="psY", name="py")
                           for _ in pair]
                    for j in range(S):
                        w11 = wslot(1 + 7 * NB + j, P, HID)
                        wout = wslot(1 + 7 * NB + S + j, HID, S)
                        gs = []
                        for c0, cs in pair:
                            ph = psH.tile([HID, CHUNK], F32, tag="psH",
                                          name="ph")
                            nc.tensor.matmul(ph[:, :cs], w11, x4[:, c0:c0 + cs],
                                             start=True, stop=True)
                            g = gbuf.tile([HID, CHUNK], F16, tag="g", name="g")
                            nc.scalar.activation(out=g[:, :cs], in_=ph[:, :cs],
                                                 func=GELU)
                            gs.append(g)
                        for py, g, (c0, cs) in zip(pys, gs, pair):
                            nc.tensor.matmul(py[:S, :cs], wout, g[:, :cs],
                                             start=(j == 0), stop=(j == S - 1))
                    for py, (c0, cs) in zip(pys, pair):
                        nc.vector.tensor_copy(out_t[:, c0:c0 + cs],
                                              py[:S, :cs])

                f_ch = _chunks(FTOT)
                for k in range(0, len(f_ch), 2):
                    yield lambda p=f_ch[k:k + 2]: emit_final_pair(p)
                yield lambda: nc.sync.dma_start(out=out_d.ap()[s],
                                                in_=out_t[:, :NBLK4])

            pending_final = None
            for s in range(BPC):
                in_t = xin.tile([8 * IN_LEN, NBLKW], F16, tag="in", name="in_t")
                nc.sync.dma_start(out=in_t[:, :1024], in_=in_d.ap()[s][:, :1024])
                nc.sync.dma_start(out=in_t[:, 1024:], in_=in_d.ap()[s][:, 1024:])

                xt = in_t
                emcount = 0
                for i in range(NB):
                    for thunk, xn in block_emissions(s, i, xt):
                        thunk()
                        emcount += 1
                        # weave the previous sample's (ACT-bound) final stage
                        # evenly across this sample's blocks (~24 block pair
                        # emissions vs 4 final thunks -> every 6th)
                        if pending_final is not None and emcount % 6 == 3:
                            th = next(pending_final, None)
                            if th is None:
                                pending_final = None
                            else:
                                th()
                    xt = xn
                pending_final = final_emissions(s, xt)

            while pending_final is not None:
                th = next(pending_final, None)
                if th is None:
                    pending_final = None
                else:
                    th()
    nc.compile()
    return nc


_NC_CACHE = {}

# Set kernel.TRACE = True (with an NTFF hook installed) to capture HW timing
# into kernel.LAST_EXEC_NS. Off by default so the kernel runs standalone.
TRACE = False
LAST_EXEC_NS = None


def _get_nc():
    if "nc" not in _NC_CACHE:
        nc = bacc.Bacc(None, target_bir_lowering=False)
        _NC_CACHE["nc"] = _emit(nc)
    return _NC_CACHE["nc"]


# ------------------------------------------------------------------- driver
def kernel(input, w_first, conv_a, conv_b, lin_m, w11, w_out, filt_d, filt_r):
    input = np.asarray(input, np.float32)
    wall = _build_wall(np.asarray(w_first, np.float32),
                       np.asarray(conv_a, np.float32),
                       np.asarray(conv_b, np.float32),
                       np.asarray(lin_m, np.float32),
                       np.asarray(w11, np.float32),
                       np.asarray(w_out, np.float32),
                       np.asarray(filt_d, np.float32),
                       np.asarray(filt_r, np.float32))

    xpad = np.pad(input, ((0, 0), (0, 0), (RECEPT, RECEPT + R_PAD)),
                  mode="wrap")
    in80 = _make_in80(xpad)  # (B, 80, NBLK0)

    nc = _get_nc()
    wall16 = wall.astype(np.float16)
    in80_16 = in80.astype(np.float16)
    in_maps = [
        {"in80": np.ascontiguousarray(in80_16[c * BPC:(c + 1) * BPC]),
         "wall": wall16}
        for c in range(N_CORES)
    ]
    import time as _time
    res = None
    for attempt in range(3):
        try:
            res = run_bass_kernel_spmd(nc, in_maps, list(range(N_CORES)),
                                       trace=TRACE)
            break
        except Exception:
            if attempt == 2:
                raise
            _time.sleep(45)
    global LAST_EXEC_NS
    LAST_EXEC_NS = res.exec_time_ns

    out = np.empty((B, 1, L), np.float32)
    for c in range(N_CORES):
        y = res.results[c]["yout"]  # (BPC, 6, NBLK4)
        flat = y.transpose(0, 2, 1).reshape(BPC, S * NBLK4)
        out[c * BPC:(c + 1) * BPC, 0, :] = flat[:, :L]
    return out


# revision 17
# speedup vs baseline: 1.0593x; 1.0593x over previous
"""Trainium2 Bass kernel for nn_NetBurgers1D_InN_legendre.

Strategy
--------
Data-parallel over batch: 8 cores x 4 samples each. All on-chip compute uses a
"phase-major" layout: SBUF row = phase*20 + channel, where phase = t mod 6 and
the free dim indexes 6-sample blocks u (t = 6u + phase). In this layout every
op of the network becomes a dense matmul on the 120-partition contraction dim:

 - first conv (10ch k=3)            : 1 matmul, lhsT (80, 120), rhs = 8-phase
                                      de-interleaved input (built on host)
 - residual conv3 -> gelu -> conv3  : 2 + 2 matmuls (block-shift taps), built
                                      with output shifts so the final add is a
                                      pure PSUM accumulation
 - Legendre decompose/mix/reconstruct: folded on host into three 120x120
                                      block-diagonal matrices (3 matmuls)
 - w11 (20->128, per phase)         : 6 zero-masked (120,128) matmuls
 - w_out (128->1, per phase)        : 6 accumulating (128,6) matmuls writing
                                      each phase to its own output partition
Matmuls run in fp16 (full-rate PE, hidden weight loads). Gelu on ScalarE reading PSUM directly;
the rec + linear_x add happens inside PSUM accumulation for free.

Host pre/post: wrap-pad + 8-phase de-interleave of the input, folding of all
weights into lhsT matrices, and re-interleave of the (6, nblk) phase-major
output back to (1, 16384).
"""

import sys

sys.path.insert(0, "/opt/trn_rl_repo")

import numpy as np

import concourse.bacc as bacc
import concourse.mybir as mybir
import concourse.tile as tile
from concourse.bass_utils import run_bass_kernel_spmd

# ---------------------------------------------------------------- constants
B, IN_LEN, L = 32, 10, 16384
N_POLY, M_MODES, K_STR, CH, NB = 12, 6, 2, 20, 4
S = N_POLY // K_STR        # 6: block size (= conv stride)
RECEPT = 25
R_PAD = 2
LPAD = L + 2 * RECEPT + R_PAD   # 16436
NBLK0 = (LPAD - 2) // S         # 2739 blocks after the first conv
NBLK4 = NBLK0 - 2 * NB          # 2731 blocks after the 4 residual blocks
# fp32r matmuls need even N, and shifted taps read past the computed range,
# so every stage computes a few junk-but-initialized spare columns. Widths
# cascade backward from the output: final reads 2732, block i consumes 2 more
# per tap level. The input is zero-padded to NBLKW on the host.
NBLKW = NBLK0 + 9               # 2748: tile width / input width
ATOT = [2746 - 4 * i for i in range(NB)]   # a-stage computed cols, block i
OTOT = [2744 - 4 * i for i in range(NB)]   # o-stage computed cols, block i
FTOT = 2732                                # final-stage computed cols
N_CORES = 8
BPC = B // N_CORES              # 4 samples per core
P = S * CH                      # 120 partitions in phase-major layout
HID = 128                       # w11 hidden width
CHUNK = 512                     # PSUM bank = 512 fp32

F32 = mybir.dt.float32
F32R = mybir.dt.float32r
F16 = mybir.dt.float16
GELU = mybir.ActivationFunctionType.Gelu

# weight wall slots: 1 first conv + 7 per block * 4 + 6 w11 + 6 w_out
N_WSLOT = 1 + 7 * NB + 2 * S
WCOL = 128


# ------------------------------------------------------------- host folding
def _conv_pair(w, shift):
    """(T0, T1) lhsT for conv3 with output shift; row jj*20+ci, col j*20+co."""
    cout, cin, kw = w.shape
    T0 = np.zeros((S * cin, S * cout), np.float32)
    T1 = np.zeros((S * cin, S * cout), np.float32)
    for j in range(S):
        for k in range(kw):
            jj = j + shift + k
            T = T0 if jj < S else T1
            T[(jj % S) * cin:(jj % S) * cin + cin,
              j * cout:j * cout + cout] += w[:, :, k].T
    return T0, T1


def _first_conv_mat(w_first):
    W1 = np.zeros((8 * IN_LEN, P), np.float32)
    for j in range(S):
        for k in range(3):
            jj = j + k
            W1[jj * IN_LEN:(jj + 1) * IN_LEN, j * CH:(j + 1) * CH] += \
                w_first[:, :, k].T
    return W1


def _legendre_mats(lin_m_i, fd, fr):
    """Fold decompose -> (mode-major grouped) mix -> reconstruct into three
    120x120 lhsT tap matrices.

    The reference flattens (mode, channel) mode-major and applies the grouped
    1x1 conv over *that* axis in chunks of 6 -- the groups straddle channels.
    So: D maps phase-major input blocks to the mode-major 120-vector, M is
    block-diagonal in mode-major flat index, R maps back per channel.
    """
    Fd0 = fd[:, 0, :S] / K_STR   # (mode, in-phase)
    Fd1 = fd[:, 0, S:] / K_STR
    Fr0 = fr[:, 0, :S] / K_STR   # (mode, out-phase)
    Fr1 = fr[:, 0, S:] / K_STR
    D0 = np.zeros((P, P), np.float32)  # [mode*20+ch, jj*20+ci]
    D1 = np.zeros_like(D0)
    R0 = np.zeros((P, P), np.float32)  # [j*20+co, mode*20+ch]
    R1 = np.zeros_like(R0)
    for ch in range(CH):
        for mode in range(M_MODES):
            f = mode * CH + ch
            for j in range(S):
                D0[f, j * CH + ch] = Fd0[mode, j]
                D1[f, j * CH + ch] = Fd1[mode, j]
                R0[j * CH + ch, f] = Fr0[mode, j]
                R1[j * CH + ch, f] = Fr1[mode, j]
    M = np.zeros((P, P), np.float32)   # mode-major block-diag groups of 6
    for g in range(CH):
        M[S * g:S * g + S, S * g:S * g + S] = lin_m_i[g]   # [o, i]
    T0 = 2.0 * (R1 @ M @ D0)           # [out, in] for tap u'
    T1 = 2.0 * (R1 @ M @ D1 + R0 @ M @ D0)
    T2 = 2.0 * (R0 @ M @ D1)
    return T0.T, T1.T, T2.T            # lhsT convention [in, out]


def _build_wall(w_first, conv_a, conv_b, lin_m, w11, w_out, filt_d, filt_r):
    """Pack every lhsT into one (128, N_WSLOT*128) array, slot i at col i*128."""
    wall = np.zeros((HID, N_WSLOT * WCOL), np.float32)
    W1 = _first_conv_mat(w_first)
    mats = [np.zeros((1, 1), np.float32)]   # slot 0 unused (conv folded away)
    for i in range(NB):
        A0, A1 = _conv_pair(conv_a[i], 3)
        B0, B1 = _conv_pair(conv_b[i], 1)
        C0, C1, C2 = _legendre_mats(lin_m[i], filt_d, filt_r)
        if i == 0:
            # block 0 reads the 8-phase input directly: x0 = W1^T in80, and
            # every block-0 lhsT T becomes W1 @ T (conv_b reads `a`, unchanged)
            A0, A1, C0, C1, C2 = (W1 @ A0, W1 @ A1, W1 @ C0, W1 @ C1, W1 @ C2)
        mats += [A0, A1, B0, B1, C0, C1, C2]
    for j in range(S):  # w11, zero-masked to phase j: (120, 128)
        Wz = np.zeros((P, HID), np.float32)
        Wz[j * CH:(j + 1) * CH, :] = w11[:, :, 0].T
        mats.append(Wz)
    for j in range(S):  # w_out into output partition j: (128, 6)
        Wz = np.zeros((HID, S), np.float32)
        Wz[:, j] = w_out[0, :, 0]
        mats.append(Wz)
    for i, m in enumerate(mats):
        wall[:m.shape[0], i * WCOL:i * WCOL + m.shape[1]] = m
    return wall


def _make_in80(xpad):
    """(b, 10, 16436) -> (b, 80, 2748): row jj*10+ci holds xpad[ci, 6u+jj].

    Columns >= NBLK0 (the spare tail) are zero."""
    b = xpad.shape[0]
    out = np.zeros((b, 8 * IN_LEN, NBLKW), np.float32)
    for jj in range(8):
        out[:, jj * IN_LEN:(jj + 1) * IN_LEN, :NBLK0] = \
            xpad[:, :, jj:jj + S * NBLK0:S][:, :, :NBLK0]
    return out


def _even(n):
    return n + (n & 1)


def _chunks(n):
    """Balanced even-size chunks of n (n even) columns, each <=CHUNK, >=256."""
    assert n % 2 == 0
    k = (n + CHUNK - 1) // CHUNK
    base = (n // k) & ~1
    rem = (n - base * k) // 2
    out = []
    c0 = 0
    for i in range(k):
        cs = base + (2 if i < rem else 0)
        out.append((c0, cs))
        c0 += cs
    assert c0 == n
    return out


# ------------------------------------------------------------- bass kernel
def _emit(nc):
    in_d = nc.dram_tensor("in80", [BPC, 8 * IN_LEN, NBLKW], F16,
                          kind="ExternalInput")
    wall_d = nc.dram_tensor("wall", [HID, N_WSLOT * WCOL], F16,
                            kind="ExternalInput")
    out_d = nc.dram_tensor("yout", [BPC, S, NBLK4], F32, kind="ExternalOutput")

    def wslot(i, k, m):
        return wall_sb[:k, i * WCOL:i * WCOL + m]

    with tile.TileContext(nc) as tc:
        with (
            tc.tile_pool(name="wsb", bufs=1) as wsb,
            tc.tile_pool(name="xin", bufs=2) as xin,
            tc.tile_pool(name="xbuf", bufs=6) as xbuf,
            tc.tile_pool(name="abuf", bufs=2) as abuf,
            tc.tile_pool(name="gbuf", bufs=8) as gbuf,
            tc.tile_pool(name="obuf", bufs=2) as obuf,
            tc.tile_pool(name="psA", bufs=2, space="PSUM") as psA,
            tc.tile_pool(name="psO", bufs=2, space="PSUM") as psO,
            tc.tile_pool(name="psH", bufs=2, space="PSUM") as psH,
            tc.tile_pool(name="psY", bufs=2, space="PSUM") as psY,
        ):
            wall_sb = wsb.tile([HID, N_WSLOT * WCOL], F16)
            nc.sync.dma_start(out=wall_sb[:, :8 * WCOL],
                              in_=wall_d.ap()[:, :8 * WCOL])
            nc.sync.dma_start(out=wall_sb[:, 8 * WCOL:],
                              in_=wall_d.ap()[:, 8 * WCOL:])

            def emit_a_chunk(W, kk, xt, a, c0, cs):
                A0, A1 = W
                pa = psA.tile([HID, CHUNK], F32, tag="psA", name="pa")
                nc.tensor.matmul(pa[:, :cs], A0[:kk], xt[:, c0:c0 + cs],
                                 start=True, stop=False)
                nc.tensor.matmul(pa[:, :cs], A1[:kk], xt[:, c0 + 1:c0 + 1 + cs],
                                 start=False, stop=True)
                nc.scalar.activation(out=a[:, c0:c0 + cs], in_=pa[:P, :cs],
                                     func=GELU)

            def emit_o_chunk(W, kk, xt, a, xn, c0, cs):
                B0, B1, C0, C1, C2 = W
                po = psO.tile([HID, CHUNK], F32, tag="psO", name="po")
                nc.tensor.matmul(po[:, :cs], B0, a[:, c0:c0 + cs],
                                 start=True, stop=False)
                nc.tensor.matmul(po[:, :cs], B1, a[:, c0 + 1:c0 + 1 + cs],
                                 start=False, stop=False)
                nc.tensor.matmul(po[:, :cs], C0[:kk], xt[:, c0:c0 + cs],
                                 start=False, stop=False)
                nc.tensor.matmul(po[:, :cs], C1[:kk], xt[:, c0 + 1:c0 + 1 + cs],
                                 start=False, stop=False)
                nc.tensor.matmul(po[:, :cs], C2[:kk], xt[:, c0 + 2:c0 + 2 + cs],
                                 start=False, stop=True)
                nc.scalar.activation(out=xn[:, c0:c0 + cs], in_=po[:P, :cs],
                                     func=GELU)

            def block_emissions(s, i, xt):
                """Generator of per-chunk emission thunks for residual block i,
                interleaving a/o chunks so PE-heavy o work hides a-gelu latency."""
                kk = 8 * IN_LEN if i == 0 else P
                A = (wslot(1 + 7 * i + 0, P, HID),
                     wslot(1 + 7 * i + 1, P, HID))
                O = tuple(wslot(1 + 7 * i + t, P, HID) for t in range(2, 7))
                a = abuf.tile([P, NBLKW], F16, tag="a", name="a")
                xn = xbuf.tile([P, NBLKW], F16, tag="x", name="xn")
                a_ch = _chunks(ATOT[i])
                o_ch = _chunks(OTOT[i])
                yield (lambda c=a_ch[0]: emit_a_chunk(A, kk, xt, a, *c)), xn
                for k, oc in enumerate(o_ch):
                    if k + 1 < len(a_ch):
                        yield (lambda c=a_ch[k + 1]:
                               emit_a_chunk(A, kk, xt, a, *c)), xn
                    yield (lambda c=oc: emit_o_chunk(O, kk, xt, a, xn, *c)), xn

            def final_emissions(s, x4):
                out_t = obuf.tile([S, FTOT], F32, tag="out", name="out_t")

                def emit_final_chunk(c0, cs):
                    py = psY.tile([S, CHUNK], F32, tag="psY", name="py")
                    gs = []
                    for j in range(S):
                        ph = psH.tile([HID, CHUNK], F32, tag="psH", name="ph")
                        nc.tensor.matmul(ph[:, :cs],
                                         wslot(1 + 7 * NB + j, P, HID),
                                         x4[:, c0:c0 + cs],
                                         start=True, stop=True)
                        g = gbuf.tile([HID, CHUNK], F16, tag="g", name="g")
                        nc.scalar.activation(out=g[:, :cs], in_=ph[:, :cs],
                                             func=GELU)
                        gs.append(g)
                    for j in range(S):
                        nc.tensor.matmul(py[:S, :cs],
                                         wslot(1 + 7 * NB + S + j, HID, S),
                                         gs[j][:, :cs],
                                         start=(j == 0), stop=(j == S - 1))
                    nc.vector.tensor_copy(out_t[:, c0:c0 + cs], py[:S, :cs])

                for c0, cs in _chunks(FTOT):
                    yield lambda c0=c0, cs=cs: emit_final_chunk(c0, cs)
                yield lambda: nc.sync.dma_start(out=out_d.ap()[s],
                                                in_=out_t[:, :NBLK4])

            pending_final = None
            for s in range(BPC):
                in_t = xin.tile([8 * IN_LEN, NBLKW], F16, tag="in", name="in_t")
                nc.sync.dma_start(out=in_t[:, :1024], in_=in_d.ap()[s][:, :1024])
                nc.sync.dma_start(out=in_t[:, 1024:], in_=in_d.ap()[s][:, 1024:])

                xt = in_t
                emcount = 0
                for i in range(NB):
                    for thunk, xn in block_emissions(s, i, xt):
                        thunk()
                        emcount += 1
                        # weave the previous sample's (ACT-bound) final stage
                        # evenly across this sample's blocks (~52 block chunk
                        # emissions vs 7 final thunks -> every 7th)
                        if pending_final is not None and emcount % 7 == 3:
                            th = next(pending_final, None)
                            if th is None:
                                pending_final = None
                            else:
                                th()
                    xt = xn
                pending_final = final_emissions(s, xt)

            while pending_final is not None:
                th = next(pending_final, None)
                if th is None:
                    pending_final = None
                else:
                    th()
    nc.compile()
    return nc


_NC_CACHE = {}

# Set kernel.TRACE = True (with an NTFF hook installed) to capture HW timing
# into kernel.LAST_EXEC_NS. Off by default so the kernel runs standalone.
TRACE = False
LAST_EXEC_NS = None


def _get_nc():
    if "nc" not in _NC_CACHE:
        nc = bacc.Bacc(None, target_bir_lowering=False)
        _NC_CACHE["nc"] = _emit(nc)
    return _NC_CACHE["nc"]


# ------------------------------------------------------------------- driver
def kernel(input, w_first, conv_a, conv_b, lin_m, w11, w_out, filt_d, filt_r):
    input = np.asarray(input, np.float32)
    wall = _build_wall(np.asarray(w_first, np.float32),
                       np.asarray(conv_a, np.float32),
                       np.asarray(conv_b, np.float32),
                       np.asarray(lin_m, np.float32),
                       np.asarray(w11, np.float32),
                       np.asarray(w_out, np.float32),
                       np.asarray(filt_d, np.float32),
                       np.asarray(filt_r, np.float32))

    xpad = np.pad(input, ((0, 0), (0, 0), (RECEPT, RECEPT + R_PAD)),
                  mode="wrap")
    in80 = _make_in80(xpad)  # (B, 80, NBLK0)

    nc = _get_nc()
    wall16 = wall.astype(np.float16)
    in80_16 = in80.astype(np.float16)
    in_maps = [
        {"in80": np.ascontiguousarray(in80_16[c * BPC:(c + 1) * BPC]),
         "wall": wall16}
        for c in range(N_CORES)
    ]
    import time as _time
    res = None
    for attempt in range(3):
        try:
            res = run_bass_kernel_spmd(nc, in_maps, list(range(N_CORES)),
                                       trace=TRACE)
            break
        except Exception:
            if attempt == 2:
                raise
            _time.sleep(45)
    global LAST_EXEC_NS
    LAST_EXEC_NS = res.exec_time_ns

    out = np.empty((B, 1, L), np.float32)
    for c in range(N_CORES):
        y = res.results[c]["yout"]  # (BPC, 6, NBLK4)
        flat = y.transpose(0, 2, 1).reshape(BPC, S * NBLK4)
        out[c * BPC:(c + 1) * BPC, 0, :] = flat[:, :L]
    return out


# revision 18
# speedup vs baseline: 1.0810x; 1.0205x over previous
"""Trainium2 Bass kernel for nn_NetBurgers1D_InN_legendre.

Strategy
--------
Data-parallel over batch: 8 cores x 4 samples each. All on-chip compute uses a
"phase-major" layout: SBUF row = phase*20 + channel, where phase = t mod 6 and
the free dim indexes 6-sample blocks u (t = 6u + phase). In this layout every
op of the network becomes a dense matmul on the 120-partition contraction dim:

 - first conv (10ch k=3)            : 1 matmul, lhsT (80, 120), rhs = 8-phase
                                      de-interleaved input (built on host)
 - residual conv3 -> gelu -> conv3  : 2 + 2 matmuls (block-shift taps), built
                                      with output shifts so the final add is a
                                      pure PSUM accumulation
 - Legendre decompose/mix/reconstruct: folded on host into three 120x120
                                      block-diagonal matrices (3 matmuls)
 - w11 (20->128, per phase)         : 6 zero-masked (120,128) matmuls
 - w_out (128->1, per phase)        : 6 accumulating (128,6) matmuls writing
                                      each phase to its own output partition
Matmuls run in fp16 (full-rate PE, hidden weight loads). Gelu on ScalarE reading PSUM directly;
the rec + linear_x add happens inside PSUM accumulation for free.

Host pre/post: wrap-pad + 8-phase de-interleave of the input, folding of all
weights into lhsT matrices, and re-interleave of the (6, nblk) phase-major
output back to (1, 16384).
"""

import sys

sys.path.insert(0, "/opt/trn_rl_repo")

import numpy as np

import concourse.bacc as bacc
import concourse.mybir as mybir
import concourse.tile as tile
from concourse.bass_utils import run_bass_kernel_spmd

# ---------------------------------------------------------------- constants
B, IN_LEN, L = 32, 10, 16384
N_POLY, M_MODES, K_STR, CH, NB = 12, 6, 2, 20, 4
S = N_POLY // K_STR        # 6: block size (= conv stride)
RECEPT = 25
R_PAD = 2
LPAD = L + 2 * RECEPT + R_PAD   # 16436
NBLK0 = (LPAD - 2) // S         # 2739 blocks after the first conv
NBLK4 = NBLK0 - 2 * NB          # 2731 blocks after the 4 residual blocks
# fp32r matmuls need even N, and shifted taps read past the computed range,
# so every stage computes a few junk-but-initialized spare columns. Widths
# cascade backward from the output: final reads 2732, block i consumes 2 more
# per tap level. The input is zero-padded to NBLKW on the host.
NBLKW = NBLK0 + 9               # 2748: tile width / input width
ATOT = [2746 - 4 * i for i in range(NB)]   # a-stage computed cols, block i
OTOT = [2744 - 4 * i for i in range(NB)]   # o-stage computed cols, block i
FTOT = 2732                                # final-stage computed cols
N_CORES = 8
BPC = B // N_CORES              # 4 samples per core
P = S * CH                      # 120 partitions in phase-major layout
HID = 128                       # w11 hidden width
CHUNK = 512                     # PSUM bank = 512 fp32

F32 = mybir.dt.float32
F32R = mybir.dt.float32r
F16 = mybir.dt.float16
GELU = mybir.ActivationFunctionType.Gelu

# weight wall slots: 1 first conv + 7 per block * 4 + 6 w11 + 6 w_out
N_WSLOT = 1 + 7 * NB + 2 * S
WCOL = 128


# ------------------------------------------------------------- host folding
def _conv_pair(w, shift):
    """(T0, T1) lhsT for conv3 with output shift; row jj*20+ci, col j*20+co."""
    cout, cin, kw = w.shape
    T0 = np.zeros((S * cin, S * cout), np.float32)
    T1 = np.zeros((S * cin, S * cout), np.float32)
    for j in range(S):
        for k in range(kw):
            jj = j + shift + k
            T = T0 if jj < S else T1
            T[(jj % S) * cin:(jj % S) * cin + cin,
              j * cout:j * cout + cout] += w[:, :, k].T
    return T0, T1


def _first_conv_mat(w_first):
    W1 = np.zeros((8 * IN_LEN, P), np.float32)
    for j in range(S):
        for k in range(3):
            jj = j + k
            W1[jj * IN_LEN:(jj + 1) * IN_LEN, j * CH:(j + 1) * CH] += \
                w_first[:, :, k].T
    return W1


def _legendre_mats(lin_m_i, fd, fr):
    """Fold decompose -> (mode-major grouped) mix -> reconstruct into three
    120x120 lhsT tap matrices.

    The reference flattens (mode, channel) mode-major and applies the grouped
    1x1 conv over *that* axis in chunks of 6 -- the groups straddle channels.
    So: D maps phase-major input blocks to the mode-major 120-vector, M is
    block-diagonal in mode-major flat index, R maps back per channel.
    """
    Fd0 = fd[:, 0, :S] / K_STR   # (mode, in-phase)
    Fd1 = fd[:, 0, S:] / K_STR
    Fr0 = fr[:, 0, :S] / K_STR   # (mode, out-phase)
    Fr1 = fr[:, 0, S:] / K_STR
    D0 = np.zeros((P, P), np.float32)  # [mode*20+ch, jj*20+ci]
    D1 = np.zeros_like(D0)
    R0 = np.zeros((P, P), np.float32)  # [j*20+co, mode*20+ch]
    R1 = np.zeros_like(R0)
    for ch in range(CH):
        for mode in range(M_MODES):
            f = mode * CH + ch
            for j in range(S):
                D0[f, j * CH + ch] = Fd0[mode, j]
                D1[f, j * CH + ch] = Fd1[mode, j]
                R0[j * CH + ch, f] = Fr0[mode, j]
                R1[j * CH + ch, f] = Fr1[mode, j]
    M = np.zeros((P, P), np.float32)   # mode-major block-diag groups of 6
    for g in range(CH):
        M[S * g:S * g + S, S * g:S * g + S] = lin_m_i[g]   # [o, i]
    T0 = 2.0 * (R1 @ M @ D0)           # [out, in] for tap u'
    T1 = 2.0 * (R1 @ M @ D1 + R0 @ M @ D0)
    T2 = 2.0 * (R0 @ M @ D1)
    return T0.T, T1.T, T2.T            # lhsT convention [in, out]


def _build_wall(w_first, conv_a, conv_b, lin_m, w11, w_out, filt_d, filt_r):
    """Pack every lhsT into one (128, N_WSLOT*128) array, slot i at col i*128."""
    wall = np.zeros((HID, N_WSLOT * WCOL), np.float32)
    W1 = _first_conv_mat(w_first)
    mats = [np.zeros((1, 1), np.float32)]   # slot 0 unused (conv folded away)
    for i in range(NB):
        A0, A1 = _conv_pair(conv_a[i], 3)
        B0, B1 = _conv_pair(conv_b[i], 1)
        C0, C1, C2 = _legendre_mats(lin_m[i], filt_d, filt_r)
        if i == 0:
            # block 0 reads the 8-phase input directly: x0 = W1^T in80, and
            # every block-0 lhsT T becomes W1 @ T (conv_b reads `a`, unchanged)
            A0, A1, C0, C1, C2 = (W1 @ A0, W1 @ A1, W1 @ C0, W1 @ C1, W1 @ C2)
        mats += [A0, A1, B0, B1, C0, C1, C2]
    for j in range(S):  # w11, zero-masked to phase j: (120, 128)
        Wz = np.zeros((P, HID), np.float32)
        Wz[j * CH:(j + 1) * CH, :] = w11[:, :, 0].T
        mats.append(Wz)
    for j in range(S):  # w_out into output partition j: (128, 6)
        Wz = np.zeros((HID, S), np.float32)
        Wz[:, j] = w_out[0, :, 0]
        mats.append(Wz)
    for i, m in enumerate(mats):
        wall[:m.shape[0], i * WCOL:i * WCOL + m.shape[1]] = m
    return wall


def _make_in80(xpad):
    """(b, 10, 16436) -> (b, 80, 2748): row jj*10+ci holds xpad[ci, 6u+jj].

    Columns >= NBLK0 (the spare tail) are zero."""
    b = xpad.shape[0]
    out = np.zeros((b, 8 * IN_LEN, NBLKW), np.float32)
    for jj in range(8):
        out[:, jj * IN_LEN:(jj + 1) * IN_LEN, :NBLK0] = \
            xpad[:, :, jj:jj + S * NBLK0:S][:, :, :NBLK0]
    return out


def _even(n):
    return n + (n & 1)


def _chunks(n):
    """Balanced even-size chunks of n (n even) columns, each <=CHUNK, >=256."""
    assert n % 2 == 0
    k = (n + CHUNK - 1) // CHUNK
    base = (n // k) & ~1
    rem = (n - base * k) // 2
    out = []
    c0 = 0
    for i in range(k):
        cs = base + (2 if i < rem else 0)
        out.append((c0, cs))
        c0 += cs
    assert c0 == n
    return out


# ------------------------------------------------------------- bass kernel
def _emit(nc):
    in_d = nc.dram_tensor("in80", [BPC, 8 * IN_LEN, NBLKW], F16,
                          kind="ExternalInput")
    wall_d = nc.dram_tensor("wall", [HID, N_WSLOT * WCOL], F16,
                            kind="ExternalInput")
    out_d = nc.dram_tensor("yout", [BPC, S, NBLK4], F32, kind="ExternalOutput")

    def wslot(i, k, m):
        return wall_sb[:k, i * WCOL:i * WCOL + m]

    with tile.TileContext(nc) as tc:
        with (
            tc.tile_pool(name="wsb", bufs=1) as wsb,
            tc.tile_pool(name="xin", bufs=2) as xin,
            tc.tile_pool(name="xbuf", bufs=6) as xbuf,
            tc.tile_pool(name="abuf", bufs=2) as abuf,
            tc.tile_pool(name="gbuf", bufs=8) as gbuf,
            tc.tile_pool(name="obuf", bufs=2) as obuf,
            tc.tile_pool(name="psA", bufs=2, space="PSUM") as psA,
            tc.tile_pool(name="psO", bufs=2, space="PSUM") as psO,
            tc.tile_pool(name="psH", bufs=2, space="PSUM") as psH,
            tc.tile_pool(name="psY", bufs=2, space="PSUM") as psY,
        ):
            wall_sb = wsb.tile([HID, N_WSLOT * WCOL], F16)
            nc.sync.dma_start(out=wall_sb[:, :8 * WCOL],
                              in_=wall_d.ap()[:, :8 * WCOL])

            def emit_a_chunk(W, kk, xt, a, c0, cs):
                A0, A1 = W
                pa = psA.tile([HID, CHUNK], F32, tag="psA", name="pa")
                nc.tensor.matmul(pa[:, :cs], A0[:kk], xt[:, c0:c0 + cs],
                                 start=True, stop=False)
                nc.tensor.matmul(pa[:, :cs], A1[:kk], xt[:, c0 + 1:c0 + 1 + cs],
                                 start=False, stop=True)
                nc.scalar.activation(out=a[:, c0:c0 + cs], in_=pa[:P, :cs],
                                     func=GELU)

            def emit_o_chunk(W, kk, xt, a, xn, c0, cs):
                B0, B1, C0, C1, C2 = W
                po = psO.tile([HID, CHUNK], F32, tag="psO", name="po")
                nc.tensor.matmul(po[:, :cs], C0[:kk], xt[:, c0:c0 + cs],
                                 start=True, stop=False)
                nc.tensor.matmul(po[:, :cs], C1[:kk], xt[:, c0 + 1:c0 + 1 + cs],
                                 start=False, stop=False)
                nc.tensor.matmul(po[:, :cs], C2[:kk], xt[:, c0 + 2:c0 + 2 + cs],
                                 start=False, stop=False)
                nc.tensor.matmul(po[:, :cs], B0, a[:, c0:c0 + cs],
                                 start=False, stop=False)
                nc.tensor.matmul(po[:, :cs], B1, a[:, c0 + 1:c0 + 1 + cs],
                                 start=False, stop=True)
                nc.scalar.activation(out=xn[:, c0:c0 + cs], in_=po[:P, :cs],
                                     func=GELU)

            def block_emissions(s, i, xt):
                """Generator of per-chunk emission thunks for residual block i,
                interleaving a/o chunks so PE-heavy o work hides a-gelu latency."""
                kk = 8 * IN_LEN if i == 0 else P
                A = (wslot(1 + 7 * i + 0, P, HID),
                     wslot(1 + 7 * i + 1, P, HID))
                O = tuple(wslot(1 + 7 * i + t, P, HID) for t in range(2, 7))
                a = abuf.tile([P, NBLKW], F16, tag="a", name="a")
                xn = xbuf.tile([P, NBLKW], F16, tag="x", name="xn")
                a_ch = _chunks(ATOT[i])
                o_ch = _chunks(OTOT[i])
                yield (lambda c=a_ch[0]: emit_a_chunk(A, kk, xt, a, *c)), xn
                for k, oc in enumerate(o_ch):
                    if k + 1 < len(a_ch):
                        yield (lambda c=a_ch[k + 1]:
                               emit_a_chunk(A, kk, xt, a, *c)), xn
                    yield (lambda c=oc: emit_o_chunk(O, kk, xt, a, xn, *c)), xn

            def final_emissions(s, x4):
                out_t = obuf.tile([S, FTOT], F32, tag="out", name="out_t")

                def emit_final_chunk(c0, cs):
                    py = psY.tile([S, CHUNK], F32, tag="psY", name="py")
                    gs = []
                    for j in range(S):
                        ph = psH.tile([HID, CHUNK], F32, tag="psH", name="ph")
                        nc.tensor.matmul(ph[:, :cs],
                                         wslot(1 + 7 * NB + j, P, HID),
                                         x4[:, c0:c0 + cs],
                                         start=True, stop=True)
                        g = gbuf.tile([HID, CHUNK], F16, tag="g", name="g")
                        nc.scalar.activation(out=g[:, :cs], in_=ph[:, :cs],
                                             func=GELU)
                        gs.append(g)
                    for j in range(S):
                        nc.tensor.matmul(py[:S, :cs],
                                         wslot(1 + 7 * NB + S + j, HID, S),
                                         gs[j][:, :cs],
                                         start=(j == 0), stop=(j == S - 1))
                    nc.vector.tensor_copy(out_t[:, c0:c0 + cs], py[:S, :cs])

                for c0, cs in _chunks(FTOT):
                    yield lambda c0=c0, cs=cs: emit_final_chunk(c0, cs)
                yield lambda: nc.sync.dma_start(out=out_d.ap()[s],
                                                in_=out_t[:, :NBLK4])

            pending_final = None
            for s in range(BPC):
                in_t = xin.tile([8 * IN_LEN, NBLKW], F16, tag="in", name="in_t")
                nc.sync.dma_start(out=in_t[:, :1024], in_=in_d.ap()[s][:, :1024])
                nc.sync.dma_start(out=in_t[:, 1024:], in_=in_d.ap()[s][:, 1024:])
                if s == 0:
                    # the bulk of the weight wall is only needed from block 1
                    # on -- keep it out of the HWDGE FIFO ahead of sample 0
                    nc.sync.dma_start(out=wall_sb[:, 8 * WCOL:],
                                      in_=wall_d.ap()[:, 8 * WCOL:])

                xt = in_t
                emcount = 0
                for i in range(NB):
                    for thunk, xn in block_emissions(s, i, xt):
                        thunk()
                        emcount += 1
                        # weave the previous sample's (ACT-bound) final stage
                        # evenly across this sample's blocks (~52 block chunk
                        # emissions vs 7 final thunks -> every 7th)
                        if pending_final is not None and emcount % 7 == 3:
                            th = next(pending_final, None)
                            if th is None:
                                pending_final = None
                            else:
                                th()
                    xt = xn
                pending_final = final_emissions(s, xt)

            while pending_final is not None:
                th = next(pending_final, None)
                if th is None:
                    pending_final = None
                else:
                    th()
    nc.compile()
    return nc


_NC_CACHE = {}

# Set kernel.TRACE = True (with an NTFF hook installed) to capture HW timing
# into kernel.LAST_EXEC_NS. Off by default so the kernel runs standalone.
TRACE = False
LAST_EXEC_NS = None


def _get_nc():
    if "nc" not in _NC_CACHE:
        nc = bacc.Bacc(None, target_bir_lowering=False)
        _NC_CACHE["nc"] = _emit(nc)
    return _NC_CACHE["nc"]


# ------------------------------------------------------------------- driver
def kernel(input, w_first, conv_a, conv_b, lin_m, w11, w_out, filt_d, filt_r):
    input = np.asarray(input, np.float32)
    wall = _build_wall(np.asarray(w_first, np.float32),
                       np.asarray(conv_a, np.float32),
                       np.asarray(conv_b, np.float32),
                       np.asarray(lin_m, np.float32),
                       np.asarray(w11, np.float32),
                       np.asarray(w_out, np.float32),
                       np.asarray(filt_d, np.float32),
                       np.asarray(filt_r, np.float32))

    xpad = np.pad(input, ((0, 0), (0, 0), (RECEPT, RECEPT + R_PAD)),
                  mode="wrap")
    in80 = _make_in80(xpad)  # (B, 80, NBLK0)

    nc = _get_nc()
    wall16 = wall.astype(np.float16)
    in80_16 = in80.astype(np.float16)
    in_maps = [
        {"in80": np.ascontiguousarray(in80_16[c * BPC:(c + 1) * BPC]),
         "wall": wall16}
        for c in range(N_CORES)
    ]
    import time as _time
    res = None
    for attempt in range(3):
        try:
            res = run_bass_kernel_spmd(nc, in_maps, list(range(N_CORES)),
                                       trace=TRACE)
            break
        except Exception:
            if attempt == 2:
                raise
            _time.sleep(45)
    global LAST_EXEC_NS
    LAST_EXEC_NS = res.exec_time_ns

    out = np.empty((B, 1, L), np.float32)
    for c in range(N_CORES):
        y = res.results[c]["yout"]  # (BPC, 6, NBLK4)
        flat = y.transpose(0, 2, 1).reshape(BPC, S * NBLK4)
        out[c * BPC:(c + 1) * BPC, 0, :] = flat[:, :L]
    return out
